# revision 16
# baseline (speedup 1.0000x reference)
"""Multi-head attention (B=4, S=2048, D=1024, H=16) on 8 Trainium2 cores.

Sharding: core c -> (batch b = c//2, head-group g = c%2). Each core computes
8 heads of one batch: QKV projections restricted to its 512 output columns,
attention, and a partial out-projection (512 of the 1024 contraction rows).
Host sums the two head-group partials per batch and adds bo.

v2: fully software-pipelined single-phase schedule. The ScalarE exp stream
(256 x [128,1024] activations ~= 284us busy) is the hard bottleneck, so the
kernel keeps it saturated end-to-end:
  - minimal prelude: only the Q/K projection chunks pair 0 needs for its
    first q-half run before the first scores matmul; every other QKV
    projection chunk is a single-PSUM-bank 8-matmul group woven between
    attention kt-steps in the PE queue (the PE has ~1us/step of slack
    under the 2.3us/step exp stream).
  - q-half-outer loop: after q-half 0 finishes on all pairs, its
    normalization + out-projection + output DMA weave into q-half 1's
    stream; only q-half 1's copy of that work remains as the tail.
  - scores matmuls are hp-interleaved so the two 64-row head tiles run
    concurrently in the PE array (row tiling); ctx matmuls col-pack the
    head pair; softmax denominators accumulate via ones-column matmuls
    (tile_position row packing) into one spare PSUM bank.
  - flushes (ctx+denominator matmuls) lag the exp stream through a pend
    queue and are gated on the V-projection group for their k-tile having
    been emitted (the PE queue is FIFO; consuming V before its producer
    is in the queue would deadlock).
PSUM: sp ping-pong 2x[128,1024] (4 banks) + sacc 1 + {prelude 3 | cacc 2 +
proj/out 1}.
"""

import sys

sys.path.insert(0, "/opt/trn_rl_repo")

import numpy as np

import concourse.bass as bass
import concourse.tile as tile
from concourse import bacc, mybir

f32 = mybir.dt.float32
f16 = mybir.dt.float16
AF = mybir.ActivationFunctionType

B = 4
S = 2048
D = 1024
DK = 64
H = 16
G = 2
NH = H // G        # 8 heads per core
EG = NH * DK       # 512 projection columns per core
N_CORES = 8

DT = D // 128      # 8 contraction d-tiles
NP = NH // 2       # 4 head pairs (= e-tiles of Q/K)
KT = S // 128      # 16 k tiles
SW = 1024          # q-half width
NQH = S // SW      # 2 q halves
QW = 512           # matmul moving width / PSUM bank width (f32)
CPH = SW // QW     # 2 q-chunks per half
NE8 = D // 128     # 8 out-projection row blocks

_TRACE = False
_NC_CACHE = {}


def _emit(tc, aps):
    nc = tc.nc
    import contextlib

    wqT, wkT, wvT, woT = aps["wqT"], aps["wkT"], aps["wvT"], aps["woT"]
    bq_, bk_, bv_ = aps["bq_"], aps["bk_"], aps["bv_"]
    outT = aps["outT"]

    xqr = aps["xqT"].rearrange("(dt p) s -> p dt s", p=128)
    xkr = aps["xkT"].rearrange("(dt p) s -> p dt s", p=128)
    xvr = aps["xvT"].rearrange("(dt p) s -> p dt s", p=128)

    with contextlib.ExitStack() as ctx:
        consts = ctx.enter_context(tc.tile_pool(name="consts", bufs=1))
        wres = ctx.enter_context(tc.tile_pool(name="wres", bufs=1))
        big = ctx.enter_context(tc.tile_pool(name="big", bufs=1))
        xstg = ctx.enter_context(tc.tile_pool(name="xstg", bufs=3))
        vstg = ctx.enter_context(tc.tile_pool(name="vstg", bufs=3))
        expp = ctx.enter_context(tc.tile_pool(name="expp", bufs=12))
        rbp = ctx.enter_context(tc.tile_pool(name="rbp", bufs=2))
        outp = ctx.enter_context(tc.tile_pool(name="outp", bufs=3))
        denp = ctx.enter_context(tc.tile_pool(name="denp", bufs=4))
        pacc = ctx.enter_context(tc.tile_pool(name="pacc", bufs=16))

        # ---- resident weights (f16); issue order matches first use ----
        wq_sb = wres.tile([128, DT, EG], f16, tag="wq")
        wk_sb = wres.tile([128, DT, EG], f16, tag="wk")
        wv_sb = wres.tile([128, DT, EG], f16, tag="wv")
        wo_sb = wres.tile([128, NP, D], f16, tag="wo")
        wqr_ = wqT.rearrange("(dt p) e -> p dt e", p=128)
        wkr_ = wkT.rearrange("(dt p) e -> p dt e", p=128)
        # pair-0 slices only: keeps the serial DMA critical path to ~3.5MB
        # before the first matmul; the rest streams in under the window.
        nc.sync.dma_start(wq_sb[:, :, 0:128], wqr_[:, :, 0:128])
        nc.sync.dma_start(wk_sb[:, :, 0:128], wkr_[:, :, 0:128])

        # ---- biases / ones ----
        sb_bq = consts.tile([128, NP], f32)
        sb_bk = consts.tile([128, NP], f32)
        sb_bv = consts.tile([128, EG], f32)
        nc.sync.dma_start(sb_bq[:], bq_.rearrange("(t p) -> p t", p=128))
        nc.sync.dma_start(sb_bk[:], bk_.rearrange("(t p) -> p t", p=128))
        ones32 = consts.tile([128, 64], f32)
        ones_all = consts.tile([128, 64], f16)
        nc.vector.memset(ones32[:], 1.0)
        nc.vector.tensor_copy(ones_all[:], ones32[:])

        def late_dmas():
            bv_bc = bass.AP(tensor=bv_.tensor, offset=bv_.offset,
                            ap=[[0, 128]] + list(bv_.ap))
            nc.sync.dma_start(sb_bv[:], bv_bc)
            nc.sync.dma_start(wv_sb[:],
                              wvT.rearrange("(dt p) e -> p dt e", p=128))
            nc.sync.dma_start(wq_sb[:, :, 128:EG], wqr_[:, :, 128:EG])
            nc.sync.dma_start(wk_sb[:, :, 128:EG], wkr_[:, :, 128:EG])
            nc.sync.dma_start(wo_sb[:],
                              woT.rearrange("(t p) e -> p t e", p=128))

        # ---- resident activations ----
        QT = big.tile([128, NP, S], f16, tag="QT")
        KTt = big.tile([128, NP, S], f16, tag="KT")
        V = big.tile([128, KT, EG], f16, tag="V")
        ctxT = big.tile([128, NP, S], f16, tag="ctxT")

        state = {"psS": None, "psC": None, "psP": None, "red": None,
                 "v_done": 0}
        pend = []

        # ================= weavable work groups =================
        # Work items carry a DMA part (issued one weave-slot early so the
        # input tile lands before the PE reaches the matmuls) and an MM part.
        class WItem:
            __slots__ = ("dma", "mm", "fetched")

            def __init__(self, dma, mm):
                self.dma, self.mm, self.fetched = dma, mm, False

            def fetch(self):
                if not self.fetched:
                    if self.dma is not None:
                        self.dma()
                    self.fetched = True

        def weave_pop(wq):
            it = wq.pop(0)
            it.fetch()
            if wq:
                wq[0].fetch()
            it.mm()

        def qk_item(pool_ref, tag, which, t, sc):
            xr, w_sb, bias, dst = (
                (xqr, wq_sb, sb_bq, QT) if which == "q"
                else (xkr, wk_sb, sb_bk, KTt))
            box = {}

            def dma():
                xt = xstg.tile([128, DT, QW], f16, tag="xt", name="xt")
                nc.sync.dma_start(xt[:], xr[:, :, sc * QW:(sc + 1) * QW])
                box["xt"] = xt

            def mm():
                xt = box["xt"]
                ps = pool_ref().tile([128, QW], f32, tag=tag, name="pp")
                for dd in range(DT):
                    nc.tensor.matmul(
                        ps[:], w_sb[:, dd, t * 128:(t + 1) * 128],
                        xt[:, dd, :],
                        start=(dd == 0), stop=(dd == DT - 1))
                nc.vector.tensor_scalar_add(
                    dst[:, t, sc * QW:(sc + 1) * QW], ps[:], bias[:, t:t + 1])

            return WItem(dma, mm)

        def qk_halves(pool_ref, tag, which, t, sc):
            """qk group split into two weave slots (d 0-3, d 4-7). The PSUM
            bank stays held between the halves, so nothing else may allocate
            from the same pool between them."""
            xr, w_sb, bias, dst = (
                (xqr, wq_sb, sb_bq, QT) if which == "q"
                else (xkr, wk_sb, sb_bk, KTt))
            box = {}

            def dma():
                xt = xstg.tile([128, DT, QW], f16, tag="xt", name="xt")
                nc.sync.dma_start(xt[:], xr[:, :, sc * QW:(sc + 1) * QW])
                box["xt"] = xt

            def mm_a():
                box["ps"] = pool_ref().tile([128, QW], f32, tag=tag,
                                            name="pp")
                for dd in range(DT // 2):
                    nc.tensor.matmul(
                        box["ps"][:], w_sb[:, dd, t * 128:(t + 1) * 128],
                        box["xt"][:, dd, :], start=(dd == 0), stop=False)

            def mm_b():
                for dd in range(DT // 2, DT):
                    nc.tensor.matmul(
                        box["ps"][:], w_sb[:, dd, t * 128:(t + 1) * 128],
                        box["xt"][:, dd, :],
                        start=False, stop=(dd == DT - 1))
                nc.vector.tensor_scalar_add(
                    dst[:, t, sc * QW:(sc + 1) * QW], box["ps"][:],
                    bias[:, t:t + 1])

            return WItem(dma, mm_a), WItem(None, mm_b)

        def v_item(pool_ref, tag, kt):
            box = {}

            def dma():
                xvt = vstg.tile([128, DT, 128], f16, tag="xvt", name="xvt")
                nc.sync.dma_start(xvt[:], xvr[:, :, kt * 128:(kt + 1) * 128])
                box["xvt"] = xvt

            def mm():
                # row-split: lower/upper 64 contraction rows run as
                # concurrent 64-row PE tiles into two different PSUM banks
                # (psP and the otherwise-idle psR red bank), hiding the
                # weight loads and halving the group's stream time.
                xvt = box["xvt"]
                ps = pool_ref().tile([128, EG], f32, tag=tag, name="pp")
                red = state["red"]
                for dd in range(DT):
                    nc.tensor.matmul(ps[:], xvt[0:64, dd, :],
                                     wv_sb[0:64, dd, :],
                                     start=(dd == 0), stop=(dd == DT - 1))
                    nc.tensor.matmul(red[:], xvt[64:128, dd, :],
                                     wv_sb[64:128, dd, :],
                                     start=(dd == 0), stop=(dd == DT - 1),
                                     skip_group_check=True)
                vtmp = vstg.tile([128, EG], f32, tag="vtmp", name="vtmp",
                                 bufs=2)
                nc.vector.tensor_copy(vtmp[:], red[:])
                nc.vector.tensor_add(V[:, kt, :], ps[:], vtmp[:])
                nc.vector.tensor_add(V[:, kt, :], V[:, kt, :], sb_bv[:])
                state["v_done"] += 1

            return WItem(dma, mm)

        def out_item(pool_ref, tag, e8, sc):
            def mm():
                ps = pool_ref().tile([128, QW], f32, tag=tag, name="pp")
                for t in range(NP):
                    nc.tensor.matmul(
                        ps[:], wo_sb[:, t, e8 * 128:(e8 + 1) * 128],
                        ctxT[:, t, sc * QW:(sc + 1) * QW],
                        start=(t == 0), stop=(t == NP - 1))
                ot = outp.tile([128, QW], f32, tag="ot", name="ot")
                nc.vector.tensor_copy(ot[:], ps[:])
                nc.sync.dma_start(
                    outT[e8 * 128:(e8 + 1) * 128, sc * QW:(sc + 1) * QW],
                    ot[:])

            return WItem(None, mm)

        po_tiles = {}

        def out01_item(pool_ref, tag, e8, sc):
            """Pairs 0-1 partial of the out-projection, staged in SBUF so
            only the pairs 2-3 half remains after the last block's fin."""
            def mm():
                ps = pool_ref().tile([128, QW], f32, tag=tag, name="pp")
                for t in range(2):
                    nc.tensor.matmul(
                        ps[:], wo_sb[:, t, e8 * 128:(e8 + 1) * 128],
                        ctxT[:, t, sc * QW:(sc + 1) * QW],
                        start=(t == 0), stop=(t == 1))
                po = pacc.tile([128, QW], f16, tag="po", name="po")
                po_tiles[(e8, sc)] = po
                nc.vector.tensor_copy(po[:], ps[:])

            return WItem(None, mm)

        def out23_item(pool_ref, tag, e8, sc):
            def mm():
                ps = pool_ref().tile([128, QW], f32, tag=tag, name="pp")
                for t in range(2, NP):
                    nc.tensor.matmul(
                        ps[:], wo_sb[:, t, e8 * 128:(e8 + 1) * 128],
                        ctxT[:, t, sc * QW:(sc + 1) * QW],
                        start=(t == 2), stop=(t == NP - 1))
                ot = outp.tile([128, QW], f32, tag="ot", name="ot")
                nc.vector.tensor_add(ot[:], ps[:], po_tiles[(e8, sc)][:])
                nc.sync.dma_start(
                    outT[e8 * 128:(e8 + 1) * 128, sc * QW:(sc + 1) * QW],
                    ot[:])

            return WItem(None, mm)

        # ================= attention =================
        # Per-head scores+exp: h0's matmuls run while ACT still exps h1's
        # previous tile, so exp(h0,kt) starts the moment exp(h1,kt-1) ends.
        def emit_scores_half(t, qh, kt, hp):
            q0 = qh * SW
            sp = state["psS"].tile([128, SW], f32, tag=f"sp{hp}",
                                   name=f"sp{hp}")
            for qc in range(CPH):
                nc.tensor.matmul(
                    sp[:, qc * QW:(qc + 1) * QW],
                    KTt[hp * 64:hp * 64 + 64, t, kt * 128:(kt + 1) * 128],
                    QT[hp * 64:hp * 64 + 64, t,
                       q0 + qc * QW:q0 + (qc + 1) * QW],
                    start=True, stop=True)
            ex = expp.tile([128, SW], f16, tag="ex", name=f"ex{hp}")
            nc.scalar.activation(ex[:], sp[:], AF.Exp, scale=0.125)
            return ex

        def fin_block(blk):
            t, qh = blk["t"], blk["qh"]
            q0 = qh * SW
            for qc in range(CPH):
                nc.vector.tensor_copy(
                    ctxT[:, t, q0 + qc * QW:q0 + (qc + 1) * QW],
                    blk["cacc"][qc][:])
            # denominators: partition-reduce the DVE-accumulated den tiles
            # via ones-matmuls packed into rows 32j of the persistent red
            # bank, then 1/x on DVE, broadcast across 64 partitions with a
            # rank-1 ones matmul, and scale ctxT. No DRAM round-trip.
            red = state["red"]
            for hp in range(2):
                for qc in range(CPH):
                    j = 2 * hp + qc
                    nc.tensor.matmul(
                        red[32 * j:32 * j + 1, :],
                        ones_all[:, 0:1],
                        blk["den"][hp][:, qc * QW:(qc + 1) * QW],
                        start=True, stop=True,
                        tile_position=(0, 32 * j),
                        skip_group_check=(j > 0))
            stg = rbp.tile([97, QW], f32, tag="stg", name="stg")
            scr = rbp.tile([97, QW], f32, tag="scr", name="scr")
            stg16 = rbp.tile([97, QW], f16, tag="stg16", name="stg16")
            nc.vector.tensor_copy(stg[:], red[0:97, :])
            nc.vector.reciprocal_approx_accurate(
                out=stg[:], in_=stg[:], scratch=scr[:])
            nc.vector.tensor_copy(stg16[:], stg[:])
            for qc in range(CPH):
                rb_ps = state["psC"].tile([128, QW], f32, tag="cacc",
                                          name="rb")
                for hp in range(2):
                    j = 2 * hp + qc
                    nc.tensor.matmul(
                        rb_ps[hp * 64:(hp + 1) * 64, :],
                        ones_all[32 * j:32 * j + 1, :],
                        stg16[32 * j:32 * j + 1, :],
                        start=True, stop=True,
                        tile_position=(32 * j, hp * 64),
                        skip_group_check=(hp > 0))
                nc.vector.tensor_mul(
                    ctxT[:, t, q0 + qc * QW:q0 + (qc + 1) * QW],
                    ctxT[:, t, q0 + qc * QW:q0 + (qc + 1) * QW],
                    rb_ps[:])

        def flush_one():
            blk, kt, exs = pend.pop(0)
            if blk["cacc"] is None:
                blk["cacc"] = [
                    state["psC"].tile([128, QW], f32, tag="cacc",
                                      name=f"cacc{qc}")
                    for qc in range(CPH)]
                blk["den"] = [
                    denp.tile([128, SW], f16, tag="den", name=f"den{hp}")
                    for hp in range(2)]
            t = blk["t"]
            for hp in range(2):
                for qc in range(CPH):
                    nc.tensor.matmul(
                        blk["cacc"][qc][hp * 64:(hp + 1) * 64, :],
                        V[:, kt, (2 * t + hp) * DK:(2 * t + hp + 1) * DK],
                        exs[hp][:, qc * QW:(qc + 1) * QW],
                        start=(kt == 0), stop=(kt == KT - 1),
                        skip_group_check=(hp > 0))
            for hp in range(2):
                if kt == 0:
                    nc.vector.tensor_copy(blk["den"][hp][:], exs[hp][:])
                else:
                    nc.vector.tensor_add(blk["den"][hp][:],
                                         blk["den"][hp][:], exs[hp][:])
            if kt == KT - 1:
                fin_block(blk)

        def can_flush():
            return (state["psC"] is not None and pend
                    and pend[0][1] < state["v_done"])

        # ================= schedule =================
        psS_ctx = contextlib.ExitStack()
        psS = psS_ctx.enter_context(
            tc.tile_pool(name="psS", bufs=1, space="PSUM"))
        state["psS"] = psS
        psR = psS_ctx.enter_context(
            tc.tile_pool(name="psR", bufs=1, space="PSUM"))
        state["red"] = psR.tile([128, QW], f32, tag="red", name="red")
        nc.vector.memset(state["red"][:], 1.0)
        state["psC"] = psS_ctx.enter_context(
            tc.tile_pool(name="psC", bufs=2, space="PSUM"))
        state["psP"] = psS_ctx.enter_context(
            tc.tile_pool(name="psP", bufs=1, space="PSUM"))

        def P(fn, *a):
            return fn((lambda: state["psP"]), "pp", *a)

        # prelude: exactly what (pair0, q-half0) needs before the first
        # scores matmul, then the remaining weight/bias DMAs.
        prelude = [P(qk_item, "q", 0, 0), P(qk_item, "q", 0, 1),
                   P(qk_item, "k", 0, 0), P(qk_item, "k", 0, 1)]
        while prelude:
            weave_pop(prelude)
        late_dmas()

        # deadline-slotted background work, one-ahead DMA prefetch
        slots = {}

        def at(s, item):
            slots.setdefault(s, []).append(item)

        def at2(s, pair):
            at(s, pair[0])
            at(s + 1, pair[1])

        for kt in range(KT):
            at(kt, P(v_item, kt))
        at(5, P(qk_item, "k", 0, 2))
        at(9, P(qk_item, "k", 0, 3))
        at(11, P(qk_item, "q", 1, 0))
        at(13, P(qk_item, "q", 1, 1))
        at(15, P(qk_item, "k", 1, 0))
        at2(17, P(qk_halves, "k", 1, 1))
        at2(19, P(qk_halves, "k", 1, 2))
        at2(21, P(qk_halves, "k", 1, 3))
        at2(23, P(qk_halves, "q", 2, 0))
        at2(25, P(qk_halves, "q", 2, 1))
        at2(27, P(qk_halves, "k", 2, 0))
        at2(30, P(qk_halves, "k", 2, 1))
        at2(34, P(qk_halves, "k", 2, 2))
        at2(38, P(qk_halves, "k", 2, 3))
        at2(41, P(qk_halves, "q", 3, 0))
        at2(43, P(qk_halves, "q", 3, 1))
        at2(45, P(qk_halves, "k", 3, 0))
        at2(49, P(qk_halves, "k", 3, 1))
        at2(53, P(qk_halves, "k", 3, 2))
        at2(57, P(qk_halves, "k", 3, 3))
        at2(59, P(qk_halves, "q", 0, 2))
        at2(61, P(qk_halves, "q", 0, 3))

        work = []
        step = 0
        for qh in range(NQH):
            for t in range(NP):
                blk = {"t": t, "qh": qh, "cacc": None, "den": None}
                for kt in range(KT):
                    ex0 = emit_scores_half(t, qh, kt, 0)
                    drain = 0
                    while drain < 2 and len(pend) > 2 and can_flush():
                        flush_one()
                        drain += 1
                    work.extend(slots.pop(step, ()))
                    if work:
                        weave_pop(work)
                    ex1 = emit_scores_half(t, qh, kt, 1)
                    pend.append((blk, kt, (ex0, ex1)))
                    step += 1
            # end of q-half: drain fully, then slot the next half's work
            while pend:
                flush_one()
            if qh == 0:
                s = 64
                for t_ in range(1, NP):
                    for c_ in (2, 3):
                        at2(s, P(qk_halves, "q", t_, c_))
                        s += 2
                og = [P(out_item, e8, sc)
                      for sc in range(CPH) for e8 in range(NE8)]
                for i, it in enumerate(og):
                    at(76 + 2 * i, it)
                o01 = [P(out01_item, e8, sc)
                       for sc in range(CPH, 2 * CPH) for e8 in range(NE8)]
                for i, it in enumerate(o01):
                    at(107 + i, it)

        # tail: leftovers + pairs 2-3 of q-half 1's out-projection
        for s in sorted(slots):
            work.extend(slots.pop(s))
        while work:
            weave_pop(work)
        tail = []
        for i, (e8, sc) in enumerate(
                [(e8, sc) for sc in range(CPH, 2 * CPH)
                 for e8 in range(NE8)]):
            if i % 3 == 0:
                tail.append(out23_item((lambda: state["psP"]), "pp", e8, sc))
            else:
                tail.append(out23_item((lambda: state["psC"]), "cacc",
                                       e8, sc))
        while tail:
            weave_pop(tail)
        psS_ctx.close()


def build():
    nc = bacc.Bacc("TRN2", target_bir_lowering=False, debug=False)
    aps = {}
    for nm in ("xqT", "xkT", "xvT"):
        aps[nm] = nc.dram_tensor(nm, [D, S], f16, kind="ExternalInput").ap()
    for nm in ("wqT", "wkT", "wvT"):
        aps[nm] = nc.dram_tensor(nm, [D, EG], f16, kind="ExternalInput").ap()
    aps["woT"] = nc.dram_tensor("woT", [EG, D], f16, kind="ExternalInput").ap()
    for nm in ("bq_", "bk_", "bv_"):
        aps[nm] = nc.dram_tensor(nm, [EG], f32, kind="ExternalInput").ap()
    aps["outT"] = nc.dram_tensor("outT", [D, S], f32, kind="ExternalOutput").ap()

    with tile.TileContext(nc) as tc:
        _emit(tc, aps)
    nc.compile()
    return nc


def _get_nc():
    if "full" not in _NC_CACHE:
        _NC_CACHE["full"] = build()
    return _NC_CACHE["full"]


def kernel(query, key, value, Wq, bq, Wk, bk, Wv, bv, Wo, bo):
    from concourse.bass_utils import run_bass_kernel_spmd

    query = np.asarray(query, dtype=np.float32)
    key = np.asarray(key, dtype=np.float32)
    value = np.asarray(value, dtype=np.float32)
    Wq, Wk, Wv, Wo = (np.asarray(w, dtype=np.float32) for w in (Wq, Wk, Wv, Wo))
    bq, bk, bv, bo = (np.asarray(b_, dtype=np.float32) for b_ in (bq, bk, bv, bo))

    nc = _get_nc()

    in_maps = []
    for c in range(N_CORES):
        b_i, g = divmod(c, G)
        cs = slice(g * EG, (g + 1) * EG)
        in_maps.append({
            "xqT": np.ascontiguousarray(query[b_i].T.astype(np.float16)),
            "xkT": np.ascontiguousarray(key[b_i].T.astype(np.float16)),
            "xvT": np.ascontiguousarray(value[b_i].T.astype(np.float16)),
            "wqT": np.ascontiguousarray(Wq[cs, :].T.astype(np.float16)),
            "wkT": np.ascontiguousarray(Wk[cs, :].T.astype(np.float16)),
            "wvT": np.ascontiguousarray(Wv[cs, :].T.astype(np.float16)),
            "woT": np.ascontiguousarray(Wo[:, cs].T.astype(np.float16)),
            "bq_": bq[cs].copy(),
            "bk_": bk[cs].copy(),
            "bv_": bv[cs].copy(),
        })

    kwargs = {}
    if _TRACE:
        kwargs = dict(trace=True)
    res = run_bass_kernel_spmd(nc, in_maps, core_ids=list(range(N_CORES)),
                               **kwargs)
    if _TRACE:
        kernel.last_results = res

    out = np.empty((B, S, D), np.float32)
    for b_i in range(B):
        acc = res.results[2 * b_i]["outT"].T + res.results[2 * b_i + 1]["outT"].T
        out[b_i] = acc + bo
    return out
